# revision 50
# baseline (speedup 1.0000x reference)
"""AdaptiveConv3 Trainium2 kernel.

Full model: 7-layer conv generator (3x3, BN folded on host) -> per-pixel
3x3x6 adaptive kernels (einsum over fixed basis) -> per-pixel contraction
with unfolded input patches.

Sharding: data-parallel over batch N=8, one image per NeuronCore.

Per-core pipeline (image [64, 128, 128]):
  - conv generator stack on TensorE in fp16, channel-major, row-stacked dual
    buffers so vertical tap pairs contract K=128 (6 matmul streams per tile
    vs 9); tap-major emission shares LDWEIGHTS and keeps the PE dense.
  - the basis contraction is refactored: zbt[p,(l,c)] = (x conv basis_l)
    computed DIRECTLY pixel-major on TensorE: per image row, stationary =
    x row-slices [(ky,c), 128 px], moving = constant block-diag basis
    selector [(ky,c), (l,c)]; 5 accumulating matmuls (the (ky2,kx0/kx1)
    taps pair as K=128 via a col-shifted s3 side tile) produce one
    [128, 384] PSUM tile per row — no transposes needed.
  - per-pixel stage on VectorE in pixel-major layout, one op per whole
    group (4 or 8 rows, FD up to 3072, amortizing per-op dispatch): per
    basis index l ONE tensor_tensor mult covers all 6 m-blocks via
    free-dim broadcast APs (1x mode — broadcast blocks 2x; expanding kt
    to dense via PE-transpose replication was tried and costs more PE
    than it saves DVE), then a SERIAL accumulate chain of dense fp16
    adds (2x mode) — a tree buys nothing on one serial engine and the
    chain needs only two live product tiles.
  - gen rows land 2-row-stacked (partitions 0:36 / 64:100) so one PE
    transpose + one scalar copy yield kt for a row pair.
  - wavefront uses single-band groups for the pipeline-fill phase (the
    28-tile minimum conv chain to the first gen rows) and 2-band groups
    at steady state; zbt runs one step ahead of its einsum, and the
    first 4 bands' zbt runs before the conv wavefront as PE p-state
    warmup (its s3 fills' bottoms go via sync/HWDGE so they don't queue
    behind the casting input loads on the gpsimd/SWDGE queue, whose
    desc-gen is ~1-2us per dma_start — hence 10 coarse input chunks,
    fine ones first). The row-0 border memset is top-half only — the
    bottom half's row 0 is image data and must not be order-dependent.
  - output written pixel-major [HW, (m,c)]; host reorders to NCHW.
Wall ~618us = ramp-to-first-einsum (~120us: ~21us fixed startup + the
serial conv chain at cold p-state) + the continuous DVE stream (~484us
span at ~96% occupancy, ~467us busy); PE ~490us busy under power
throttling (avg util limit 0.77-0.83; identical binaries swing ±20%
with board temperature). GpSimd offload of einsum ops REGRESSES (Pool
shares the DVE's second SBUF port; concurrent Pool tensor ops slow DVE
~1.6x); so do DRAM accum-DMA adds and deferring weight loads.
"""

from contextlib import ExitStack

import numpy as np

N, C, H, W = 8, 64, 128, 128
INTER = 64
FEAT = 6
M = 6
KS = 3
L = KS * KS          # 9
NMID = 5
GOUT = FEAT * M      # 36
OUTC = C * M         # 384
HP, WP = H + 2, W + 2          # 130
NPAD = HP * WP                 # 16900
HWTOT = H * W                  # 16384
BN_EPS = 1e-5
NBAND = 32                     # 4-row bands
RPB = 4                        # rows per band
NT = RPB * W                   # 512 free elems per conv tile

_CACHE = {}


def _build_program():
    """Build the SPMD Bass program (same graph on all 8 cores)."""
    import concourse.bacc as bacc
    import concourse.mybir as mybir
    from concourse.tile import TileContext

    fp32 = mybir.dt.float32
    bf16 = mybir.dt.float16
    AF = mybir.ActivationFunctionType
    OP = mybir.AluOpType

    nc = bacc.Bacc("TRN2", debug=False)

    # ---------------- DRAM I/O ----------------
    x_d = nc.dram_tensor("x", [C, H, W], fp32, kind="ExternalInput")
    # paired stationaries, host layout [128 (=2 ky x 64 ic), 7*3*64]
    wpair_d = nc.dram_tensor("wpair", [128, 7 * 3 * 64], bf16, kind="ExternalInput")
    # single (ky=2) stationaries [64 ic, 7*3*64]
    wsing_d = nc.dram_tensor("wsing", [64, 7 * 3 * 64], bf16, kind="ExternalInput")
    bias_d = nc.dram_tensor("bias", [64, 7], fp32, kind="ExternalInput")
    # basis selector moving operands: zselp[(ky,c'), kx, (l,c)], zsels[c', kx, (l,c)]
    zselp_d = nc.dram_tensor("zselp", [128, 3 * 384], bf16, kind="ExternalInput")
    zsels_d = nc.dram_tensor("zsels", [64, 3 * 384], bf16, kind="ExternalInput")
    # col-paired (ky=2, kx in {0,1}) selector for the s3 tiles
    zselp2_d = nc.dram_tensor("zselp2", [128, 384], bf16, kind="ExternalInput")
    ident_d = nc.dram_tensor("ident", [128, 128], bf16, kind="ExternalInput")
    out_d = nc.dram_tensor("out", [HWTOT, OUTC], bf16, kind="ExternalOutput")

    with TileContext(nc) as tc, ExitStack() as es:
        # ------------- persistent SBUF -------------
        x2 = nc.alloc_sbuf_tensor("x2", [128, NPAD], bf16)
        t2a = nc.alloc_sbuf_tensor("t2a", [128, NPAD], bf16)
        t2b = nc.alloc_sbuf_tensor("t2b", [128, NPAD], bf16)
        wpair_sb = nc.alloc_sbuf_tensor("wpair_sb", [128, 7 * 3 * 64], bf16)
        wsing_sb = nc.alloc_sbuf_tensor("wsing_sb", [64, 7 * 3 * 64], bf16)
        bias_sb = nc.alloc_sbuf_tensor("bias_sb", [64, 7], fp32)
        zselp_sb = nc.alloc_sbuf_tensor("zselp_sb", [128, 3 * 384], bf16)
        zsels_sb = nc.alloc_sbuf_tensor("zsels_sb", [64, 3 * 384], bf16)
        zselp2_sb = nc.alloc_sbuf_tensor("zselp2_sb", [128, 384], bf16)
        ident_sb = nc.alloc_sbuf_tensor("ident_sb", [128, 128], bf16)
        # gen, 2-row stacked: row pair rp, even rows at partitions 0:36,
        # odd rows at 64:100 (partition bases must be 32-aligned),
        # cols rp*128+p
        gen_sb = nc.alloc_sbuf_tensor("gen_sb", [128, HWTOT // 2], bf16)

        x2v = x2[:].rearrange("p (h w) -> p h w", h=HP, w=WP)
        t2av = t2a[:].rearrange("p (h w) -> p h w", h=HP, w=WP)
        t2bv = t2b[:].rearrange("p (h w) -> p h w", h=HP, w=WP)

        # ------------- load constants -------------
        # x (f32 -> bf16) first in the gpsimd queue (converting DMAs are
        # SWDGE-only) — nothing can start until the first rows land, and
        # SWDGE desc-gen is ~1-2us per dma_start, so the load is
        # issue-rate-bound: 10 coarse chunks instead of 32 fine ones. Top
        # half holds the padded image at rows 1..128; bottom half
        # (partitions 64:128) the same image one padded row up. Fine chunks
        # lead so conv L0 of band 0 can start after 5 rows.
        XCH = ((0, 16), (16, 16), (32, 32), (64, 32), (96, 32))
        nc.sync.dma_start(out=wpair_sb[:], in_=wpair_d[:])
        for c0, csz in XCH:
            nc.gpsimd.dma_start(out=x2v[0:64, 1 + c0:1 + c0 + csz, 1:129],
                                in_=x_d[:, c0:c0 + csz, :])
            nc.gpsimd.dma_start(out=x2v[64:128, c0:c0 + csz, 1:129],
                                in_=x_d[:, c0:c0 + csz, :])
        # Only the pad borders need zeroing (interiors are fully written):
        # top/bottom pad rows, left/right pad columns, and the bottom-half's
        # two trailing rows (its interior covers padded rows 0..127 only).
        # These run on the Vector engine, which is idle through the ramp.
        for bufv in (x2v, t2av, t2bv):
            # row 0: top half only — the bottom half's row 0 is real data
            # (image row 0), written by the DMA/conv and racing with this
            nc.vector.memset(bufv[0:64, 0:1, :], 0.0)
            nc.vector.memset(bufv[:, 129:130, :], 0.0)   # padded row 129
            nc.vector.memset(bufv[64:128, 128:129, :], 0.0)  # bottom-half row 128
            nc.vector.memset(bufv[:, :, 0:1], 0.0)       # left pad col
            nc.vector.memset(bufv[:, :, 129:130], 0.0)   # right pad col
        # zero gen_sb's unwritten partition bands (read by the full-width
        # kt transposes)
        nc.vector.memset(gen_sb[32:64, :], 0.0)
        nc.vector.memset(gen_sb[96:128, :], 0.0)
        nc.sync.dma_start(out=wsing_sb[:], in_=wsing_d[:])
        nc.sync.dma_start(out=bias_sb[:], in_=bias_d[:])
        nc.sync.dma_start(out=zselp_sb[:], in_=zselp_d[:])
        nc.sync.dma_start(out=zsels_sb[:], in_=zsels_d[:])
        nc.sync.dma_start(out=zselp2_sb[:], in_=zselp2_d[:])
        nc.sync.dma_start(out=ident_sb[:], in_=ident_d[:])

        # ------------- tile pools -------------
        conv_ps = es.enter_context(tc.tile_pool(name="conv_ps", bufs=3, space="PSUM"))
        zt_ps_pool = es.enter_context(tc.tile_pool(name="zt_ps", bufs=3, space="PSUM"))
        kt_ps_pool = es.enter_context(tc.tile_pool(name="kt_ps", bufs=2, space="PSUM"))
        kt_pool = es.enter_context(tc.tile_pool(name="kt", bufs=4))
        zbt_pool = es.enter_context(tc.tile_pool(name="zbt", bufs=4))
        acc_pool = es.enter_context(tc.tile_pool(name="acc", bufs=2))
        pl_pool = es.enter_context(tc.tile_pool(name="pl", bufs=2))
        s3x_pool = es.enter_context(tc.tile_pool(name="s3x", bufs=6))

        layer_src = [x2v, t2av, t2bv, t2av, t2bv, t2av, t2bv]
        layer_dst = [t2av, t2bv, t2av, t2bv, t2av, t2bv, None]

        s3x_tiles = {}

        def s3_fill(srcv, b, pool, nm, bot_eng=None):
            """Col-shift side tile for band b: [128, RPB, 130] with
            top = src top rows r0+2..r0+5, bottom = same shifted 1 col
            left, so the (ky2,kx0)+(ky2,kx1) taps contract as K=128."""
            r0 = b * RPB
            s3 = pool.tile([128, RPB * 130], bf16, tag="s3", name=nm)
            s3v = s3[:].rearrange("p (h w) -> p h w", h=RPB, w=130)
            nc.sync.dma_start(out=s3v[0:64, :, :],
                              in_=srcv[0:64, r0 + 2:r0 + 6, 0:130])
            (bot_eng or nc.gpsimd).dma_start(
                out=s3v[64:128, :, 0:129],
                in_=srcv[0:64, r0 + 2:r0 + 6, 1:130])
            return s3v

        def conv_group(lyr, bands):
            """Conv tiles for the given bands, tap-major so each stationary
            is loaded once per group (LDWEIGHTS amortized, dense PE burst)."""
            src = layer_src[lyr]
            tiles = [conv_ps.tile([64, NT], fp32, tag="conv",
                                  name=f"cps_{lyr}_{b}") for b in bands]
            views = [t[:].rearrange("p (h w) -> p h w", h=RPB, w=W) for t in tiles]
            # paired taps: ky in {0,1}, K=128. Pairs-first matters: they
            # depend on the looser (earlier-issued) inputs — the dual-row
            # bot-copy of band b — while the singles' top rows include the
            # same-step neighbor band's activation (the tightest dep), so
            # running them last gives it time to land (singles-first
            # measured +37us).
            for kx in range(3):
                off = (lyr * 3 + kx) * 64
                for b, psv in zip(bands, views):
                    r0 = b * RPB
                    nc.tensor.matmul(psv, wpair_sb[:, off:off + 64],
                                     src[:, r0:r0 + RPB, kx:kx + W],
                                     start=(kx == 0), stop=False)
            # single taps: ky=2, K=64 (top half only)
            for kx in range(3):
                off = (lyr * 3 + kx) * 64
                for b, psv in zip(bands, views):
                    r0 = b * RPB
                    nc.tensor.matmul(psv, wsing_sb[:, off:off + 64],
                                     src[0:64, r0 + 2:r0 + 2 + RPB, kx:kx + W],
                                     start=False, stop=(kx == 2))
            for b, ps, psv in zip(bands, tiles, views):
                r0 = b * RPB
                if lyr < 6:
                    dst = layer_dst[lyr]
                    func = AF.Tanh if lyr == 0 else AF.Identity
                    top = dst[0:64, r0 + 1:r0 + 1 + RPB, 1:1 + W]
                    nc.scalar.activation(top, psv, func,
                                         bias=bias_sb[:, lyr:lyr + 1], scale=1.0)
                    bot = dst[64:128, r0:r0 + RPB, 1:1 + W]
                    nc.sync.dma_start(out=bot, in_=top)
                else:
                    # final layer -> gen (36 channels, tanh, unpadded),
                    # 2-row stacked: even rows -> partitions 0:36, odd ->
                    # 36:72, so one PE transpose covers a row pair.
                    psr = ps[0:GOUT, :].rearrange("q (rr w) -> q rr w", w=W)
                    for par in range(2):  # parity within the band
                        src = psr[:, par::2, :]
                        dst = gen_sb[par * 64:par * 64 + GOUT,
                                     (r0 // 2) * W:(r0 // 2 + 2) * W]\
                            .rearrange("q (rp w) -> q rp w", w=W)
                        nc.scalar.activation(dst, src, AF.Tanh,
                                             bias=bias_sb[0:GOUT, 6:7], scale=1.0)

        zbt_tiles = {}

        def group_zbt(bands):
            """zbt for a group (4 rows per band): basis depthwise conv,
            pixel-major [128 px, (r, l, c)] via 5 PE matmuls per row with x
            row-slices as stationaries. The (ky2,kx0)+(ky2,kx1) taps pair as
            K=128 via the col-shifted s3 side tiles."""
            g0 = bands[0]
            zbt = zbt_pool.tile([128, 8 * 384], bf16, tag="zbt",
                                name=f"zbt_{g0}")
            for b in bands:
                s3v = s3x_tiles.pop(b)
                for sub in range(4):
                    r = 4 * b + sub
                    i = r - 4 * g0
                    zt_ps = zt_ps_pool.tile([128, 384], fp32, tag="ztps",
                                            name=f"ztps_{r}")
                    for kx in range(3):
                        nc.tensor.matmul(zt_ps[:], x2v[:, r, kx:kx + W],
                                         zselp_sb[:, kx * 384:(kx + 1) * 384],
                                         start=(kx == 0), stop=False)
                    nc.tensor.matmul(zt_ps[:], s3v[:, sub, 0:W], zselp2_sb[:],
                                     start=False, stop=False)
                    nc.tensor.matmul(zt_ps[:], s3v[0:64, sub, 2:2 + W],
                                     zsels_sb[:, 2 * 384:3 * 384],
                                     start=False, stop=True)
                    nc.scalar.activation(zbt[:, i * 384:(i + 1) * 384],
                                         zt_ps[:], AF.Copy)
            zbt_tiles[g0] = zbt

        kt_tiles = {}

        def group_kt(bands):
            """kt for a group via 2-row PE transposes (gen is 2-row stacked)
            + scalar copies into a [128, (r, m, l)] tile."""
            g0 = bands[0]
            kt = kt_pool.tile([128, 8 * GOUT], bf16, tag="kt")
            ktv = kt[:].rearrange("p (rq r q) -> p rq r q", rq=4, r=2)
            for b in bands:
                for sub in range(2):
                    rp = 2 * b + sub
                    rq = rp - 2 * g0
                    kt_ps = kt_ps_pool.tile([128, 128], bf16, tag="ktps")
                    nc.tensor.transpose(kt_ps[:],
                                        gen_sb[:, rp * W:(rp + 1) * W],
                                        ident_sb[:])
                    nc.scalar.activation(
                        ktv[:, rq],
                        kt_ps[:].rearrange("p (r x) -> p r x",
                                           x=64)[:, :, 0:GOUT],
                        AF.Copy)
            kt_tiles[g0] = kt

        def group_einsum2(bands):
            """Group of 1-2 bands (4 or 8 rows, FD 1536/3072): 6 broadcast
            mults + serial accumulate chain on DVE (a tree buys nothing on
            one serial engine, and the chain needs only two live product
            tiles, so op sizes double without extra SBUF). Folding the last
            product via an accumulating SWDGE out-DMA (CCE add on DRAM) is
            correct but ~5x slower than the DVE add it replaces."""
            g0 = bands[0]
            R = 4 * len(bands)
            kt = kt_tiles.pop(g0)
            zbt = zbt_tiles.pop(g0)
            kt4 = kt[:].rearrange("p (r m l) -> p r m l", r=8, l=M)
            zb4 = zbt[:].rearrange("p (r l c) -> p r l c", r=8, c=64)
            FD = R * OUTC

            def prod(l, tag):
                gbc = kt4[:, 0:R, :, l:l + 1].to_broadcast((128, R, M, 64))
                zbc = zb4[:, 0:R, l:l + 1, :].to_broadcast((128, R, M, 64))
                pl = pl_pool.tile([128, 8 * OUTC], bf16, tag=tag)
                plv = pl[:, 0:FD].rearrange("p (r m c) -> p r m c",
                                            r=R, c=64)
                nc.vector.tensor_tensor(plv, gbc, zbc, op=OP.mult)
                return pl
            acc = acc_pool.tile([128, 8 * OUTC], bf16, tag="acc")
            pa = prod(0, "plA")
            for l in range(1, M):
                pb = prod(l, "plB" if l % 2 else "plC")
                dst = acc[:, 0:FD] if l == M - 1 else pa[:, 0:FD]
                nc.vector.tensor_tensor(dst, pa[:, 0:FD], pb[:, 0:FD],
                                        op=OP.add)
            r = 4 * g0
            nc.sync.dma_start(
                out=out_d[r * W:(r + R) * W, :]
                .rearrange("(rr p) c -> p rr c", p=128),
                in_=acc[:, 0:FD].rearrange("p (rr c) -> p rr c", rr=R))

        # ------------- wavefront emission (variable groups) -------------
        # Single-band groups for the pipeline-fill phase (so the first gen
        # rows — and with them the DVE stream — start ~3x earlier), 2-band
        # groups for the steady state. conv layer lyr processes group
        # (step - lyr). zbt depends only on x, so it runs one step AHEAD of
        # its einsum (s3x fills at step-4, zbt at step-5): within the
        # einsum's step, the DVE's last prerequisite is then the kt
        # transposes, emitted right after the convs — not the 40-matmul zbt
        # burst — so the products start earlier in the step.
        GROUPS = [[b] for b in range(6)] + \
                 [[b, b + 1] for b in range(6, NBAND, 2)]
        NGRP = len(GROUPS)
        # PE warmup: the zbt matmuls for the first EARLY groups depend only
        # on x, so they run while x still streams in — ~13us of continuous
        # PE work that lifts the p-state to full clock BEFORE the serial
        # 28-tile conv chain to the first gen rows (which otherwise runs
        # ~2x slow), and removes zbt from those early steps.
        EARLY = 4
        for g in range(EARLY):
            for b in GROUPS[g]:
                s3x_tiles[b] = s3_fill(x2v, b, s3x_pool, f"s3x_{b}",
                                       bot_eng=nc.sync)
        for g in range(EARLY):
            group_zbt(GROUPS[g])
        for step in range(NGRP + 7):
            for lyr in range(7):
                grp = step - lyr
                if 0 <= grp < NGRP:
                    conv_group(lyr, GROUPS[grp])
            gk = step - 6
            if 0 <= gk < NGRP:
                group_kt(GROUPS[gk])
            gf = step - 4
            if EARLY <= gf < NGRP:
                for b in GROUPS[gf]:
                    s3x_tiles[b] = s3_fill(x2v, b, s3x_pool, f"s3x_{b}")
            gzb = step - 5
            if EARLY <= gzb < NGRP:
                group_zbt(GROUPS[gzb])
            ge = step - 6
            if 0 <= ge < NGRP:
                group_einsum2(GROUPS[ge])

    nc.finalize()
    return nc


def _prep_inputs(inputs):
    """Host-side weight prep: BN folding, tap pairing, basis selectors."""
    bf = np.float16

    f = lambda k: np.asarray(inputs[k], np.float32)
    W0, b0, g0, be0, m0, v0 = (f(k) for k in ("W0", "b0", "g0", "be0", "m0", "v0"))
    Wmid, bmid = f("Wmid"), f("bmid")
    Wf, bf_, gf, bef, mf, vf = (f(k) for k in ("Wf", "bf", "gf", "bef", "mf", "vf"))
    bases = f("bases")

    s0 = g0 / np.sqrt(v0 + BN_EPS)
    W0p = W0 * s0[:, None, None, None]
    b0p = (b0 - m0) * s0 + be0
    sf = gf / np.sqrt(vf + BN_EPS)
    Wfp = Wf * sf[:, None, None, None]
    bfp = (bf_ - mf) * sf + bef

    # layer weights [oc, ic, ky, kx] -> paired/single stationaries
    Wf64 = np.zeros((64, 64, 3, 3), np.float32)
    Wf64[:GOUT] = Wfp
    Ws = [W0p] + [Wmid[i] for i in range(NMID)] + [Wf64]
    wpair = np.zeros((7, 3, 128, 64), np.float32)
    wsing = np.zeros((7, 3, 64, 64), np.float32)
    for lyr in range(7):
        w = Ws[lyr]
        for kx in range(3):
            wpair[lyr, kx, 0:64] = w[:, :, 0, kx].T     # ky=0 -> top partitions
            wpair[lyr, kx, 64:128] = w[:, :, 1, kx].T   # ky=1 -> bottom
            wsing[lyr, kx] = w[:, :, 2, kx].T           # ky=2

    bias = np.zeros((64, 7), np.float32)
    bias[:, 0] = b0p
    for i in range(NMID):
        bias[:, 1 + i] = bmid[i]
    bias[:GOUT, 6] = bfp

    # basis selector moving operands for the pixel-major zbt matmuls:
    # zt[p,(l,c)] = sum_{ky,c'} x2[(ky,c'), r, p+kx] * zselp[(ky,c'), kx, (l,c)]
    # so zselp[(ky,c'), kx, (l,c)] = bases[l, ky*3+kx] * delta(c,c')
    zselp = np.zeros((2, 64, 3, M, 64), np.float32)
    zsels = np.zeros((64, 3, M, 64), np.float32)
    eye = np.eye(64, dtype=np.float32)
    for kx in range(3):
        for l in range(M):
            for ky in range(2):
                zselp[ky, :, kx, l, :] = eye * bases[l, ky * 3 + kx]
            zsels[:, kx, l, :] = eye * bases[l, 6 + kx]
    zselp = zselp.reshape(128, 3 * 384)
    zsels = zsels.reshape(64, 3 * 384)
    # col-paired basis selector: top half coef bases[l, ky2 kx0], bottom
    # half bases[l, ky2 kx1]
    zselp2 = np.zeros((2, 64, M, 64), np.float32)
    for l in range(M):
        zselp2[0, :, l, :] = eye * bases[l, 6]
        zselp2[1, :, l, :] = eye * bases[l, 7]
    zselp2 = zselp2.reshape(128, 384)

    ident = np.eye(128, dtype=np.float32)

    wpair = wpair.transpose(2, 0, 1, 3).reshape(128, 7 * 3 * 64)
    wsing = wsing.transpose(2, 0, 1, 3).reshape(64, 7 * 3 * 64)
    return {
        "wpair": np.ascontiguousarray(wpair).astype(bf),
        "wsing": np.ascontiguousarray(wsing).astype(bf),
        "bias": bias,
        "zselp": np.ascontiguousarray(zselp).astype(bf),
        "zsels": np.ascontiguousarray(zsels).astype(bf),
        "zselp2": np.ascontiguousarray(zselp2).astype(bf),
        "ident": ident.astype(bf),
    }


def _env_int(name, default):
    import os
    v = os.environ.get(name)
    return default if v is None else int(v)


def _install_ntff_hook():
    """Provide antenv.axon_hooks (missing in this image) so bass_utils can
    NTFF-profile under axon via the injected libaxon_pjrt.so."""
    import sys
    import types
    if "antenv.axon_hooks" in sys.modules:
        return
    try:
        import antenv
        from trn_agent_boot.trn_boot import _ntff_profile_via_ctypes
        hook = _ntff_profile_via_ctypes("/opt/axon/libaxon_pjrt.so")
    except Exception:
        return
    mod = types.ModuleType("antenv.axon_hooks")
    holder = {"h": hook}
    mod.set_axon_ntff_profile_hook = lambda h: holder.__setitem__("h", h)
    mod.get_axon_ntff_profile_hook = lambda: holder.get("h")
    sys.modules["antenv.axon_hooks"] = mod
    antenv.axon_hooks = mod


def kernel(**inputs):
    from concourse import bass_utils

    key = "prog"
    if key not in _CACHE:
        _CACHE[key] = _build_program()
    nc = _CACHE[key]

    shared = _prep_inputs(inputs)
    x_full = np.asarray(inputs["input"], np.float32)
    in_maps = [dict(shared, x=np.ascontiguousarray(x_full[i])) for i in range(N)]

    trace = bool(_env_int("ADAPT_TRACE", 0))
    if trace:
        _install_ntff_hook()
    res = bass_utils.run_bass_kernel_spmd(
        nc, in_maps, core_ids=list(range(N)), trace=trace)
    if trace:
        _CACHE["last_result"] = res

    out = np.empty((N, OUTC, H, W), np.float32)
    for i in range(N):
        o = np.asarray(res.results[i]["out"], dtype=np.float32)  # [HW, (m,c)]
        o4 = o.reshape(H, W, M, C)
        out[i] = o4.transpose(3, 2, 0, 1).reshape(OUTC, H, W)
    return out


if __name__ == "__main__":
    import time
    t0 = time.time()
    nc = _build_program()
    print(f"program built in {time.time() - t0:.1f}s")

